# revision 5
# baseline (speedup 1.0000x reference)
"""Trainium2 Bass kernel for nn_CrossAttentionBlock (B=4, L=1024, D=1024, H=16).

Sharding: 8 cores = 4 batches x 2 token-halves. Core c handles batch c//2.
Stage 1 (cross-attn) is computed for the full batch on each core (duplicated
within the pair -> no collectives); stage 2 + MLP only for the core's own
512-token half. Per-core token order is permuted so the own half is always
tokens [0:512] locally (keeps the SPMD program identical across cores).

Layout: everything runs "d-major" (features on partitions, tokens on the free
axis), so no on-device transposes are needed anywhere. The host transposes
inputs/outputs and pre-folds LN affine params + attention scale into the
projection weights. Attention biases are applied multiplicatively after exp
(host precomputes exp(bias) in bf16). Projection biases ride the ACT drain's
per-partition bias port.
"""

import sys
import numpy as np

if "/opt/trn_rl_repo" not in sys.path:
    sys.path.insert(0, "/opt/trn_rl_repo")

import ml_dtypes  # noqa: E402

B, L, D, H, MAX = 4, 1024, 1024, 16, 1024
HD = D // H
SCALE = 1.0 / float(np.sqrt(np.float32(HD)))
P = 128
NT = D // P          # 8 d-tiles
CH = 512             # token chunk width
DH = 4 * D           # 4096
NT2 = DH // P        # 32

_CACHE = {}


def build_nc():
    import concourse.bacc as bacc
    import concourse.mybir as mybir
    import concourse.tile as tile

    f32r = mybir.dt.float32r
    f32 = mybir.dt.float32
    bf16 = mybir.dt.bfloat16
    AF = mybir.ActivationFunctionType
    ALU = mybir.AluOpType

    nc = bacc.Bacc("TRN2", target_bir_lowering=False)

    # ---- params ----
    qT = nc.declare_dram_parameter("qT", [D, L], f32r, isOutput=False)
    kT = nc.declare_dram_parameter("kT", [D, L], f32r, isOutput=False)
    vT = nc.declare_dram_parameter("vT", [D, L], f32r, isOutput=False)
    wq = nc.declare_dram_parameter("wq", [D, D], f32r, isOutput=False)
    wk = nc.declare_dram_parameter("wk", [D, D], f32r, isOutput=False)
    wv = nc.declare_dram_parameter("wv", [D, D], f32r, isOutput=False)
    wq2 = nc.declare_dram_parameter("wq2", [D, D], f32r, isOutput=False)
    wk2 = nc.declare_dram_parameter("wk2", [D, D], f32r, isOutput=False)
    wv2 = nc.declare_dram_parameter("wv2", [D, D], f32r, isOutput=False)
    wo = nc.declare_dram_parameter("wo", [D, D], f32r, isOutput=False)
    w1 = nc.declare_dram_parameter("w1", [D, DH], f32r, isOutput=False)
    w2 = nc.declare_dram_parameter("w2", [DH, DH], f32r, isOutput=False)
    w3 = nc.declare_dram_parameter("w3", [DH, D], f32r, isOutput=False)
    # biases, column layout: bcol[p, g] = b[g*128 + p]
    bqc = nc.declare_dram_parameter("bqc", [P, NT], f32, isOutput=False)
    bkc = nc.declare_dram_parameter("bkc", [P, NT], f32, isOutput=False)
    bq2c = nc.declare_dram_parameter("bq2c", [P, NT], f32, isOutput=False)
    bk2c = nc.declare_dram_parameter("bk2c", [P, NT], f32, isOutput=False)
    boc = nc.declare_dram_parameter("boc", [P, NT], f32, isOutput=False)
    b1c = nc.declare_dram_parameter("b1c", [P, NT2], f32, isOutput=False)
    b2c = nc.declare_dram_parameter("b2c", [P, NT2], f32, isOutput=False)
    b3c = nc.declare_dram_parameter("b3c", [P, NT], f32, isOutput=False)
    # V-proj biases as rows (rank-1 matmul path)
    bvr = nc.declare_dram_parameter("bvr", [1, D], f32r, isOutput=False)
    onesr = nc.declare_dram_parameter("onesr", [1, CH], f32r, isOutput=False)
    sumwc = nc.declare_dram_parameter("sumwc", [P, 1], f32r, isOutput=False)
    vones = nc.declare_dram_parameter("vones", [P, NT, H, 1], bf16, isOutput=False)
    bv2r = nc.declare_dram_parameter("bv2r", [1, D], f32r, isOutput=False)
    eb1 = nc.declare_dram_parameter("eb1", [H, L, L], bf16, isOutput=False)
    eb2 = nc.declare_dram_parameter("eb2", [H, L, CH], bf16, isOutput=False)
    outT = nc.declare_dram_parameter("outT", [D, CH], f32r, isOutput=True)

    def wre(w):  # [T*128, N] -> [p, t, n] view for DMA
        return w.rearrange("(t p) o -> p t o", p=P)

    with tile.TileContext(nc) as tc:
        from contextlib import ExitStack
        es = ExitStack()
        with es:
            constp = es.enter_context(tc.tile_pool(name="constp", bufs=1))
            rowsp = es.enter_context(tc.tile_pool(name="rowsp", bufs=1))
            bcp = es.enter_context(tc.tile_pool(name="bcp", bufs=1))
            xsqp = es.enter_context(tc.tile_pool(name="xsqp", bufs=2))
            outerp = es.enter_context(tc.tile_pool(name="outerp", bufs=1))
            psp = es.enter_context(tc.tile_pool(name="psp", bufs=7, space="PSUM"))

            # ---- constants ----
            ones512 = constp.tile([1, CH], f32r)
            nc.sync.dma_start(out=ones512, in_=onesr[:, :])
            sumw = constp.tile([P, 1], f32r)
            nc.sync.dma_start(out=sumw, in_=sumwc[:, :])
            epsT = constp.tile([1, 1], f32)
            nc.vector.memset(epsT, 1e-5)
            bcol = {}
            for nm, prm, g in [("bq", bqc, NT), ("bk", bkc, NT),
                               ("bq2", bq2c, NT), ("bk2", bk2c, NT),
                               ("bo", boc, NT), ("b1", b1c, NT2),
                               ("b2", b2c, NT2), ("b3", b3c, NT)]:
                t = constp.tile([P, g], f32, name=f"bc_{nm}")
                nc.sync.dma_start(out=t, in_=prm[:, :])
                bcol[nm] = t

            _wctr = [0]

            def ps_tile(name):
                return psp.tile([P, CH], f32, tag="mm", name=name)

            # ---------------- LayerNorm ----------------
            def layernorm(x_get, dst, n_chunks, dma_src=None):
                """dst[:, t, c*CH:...] = (x - mean_d(x)) * rsqrt(var_d(x)+eps)."""
                for c in range(n_chunks):
                    xs = []
                    sum_ps = ps_tile("ln_sum")
                    sq_ps = ps_tile("ln_sq")
                    for t in range(NT):
                        x = x_get(t, c)
                        if dma_src is not None:
                            nc.sync.dma_start(
                                out=x, in_=dma_src[t * P:(t + 1) * P, c * CH:(c + 1) * CH])
                        xs.append(x)
                        xsq = xsqp.tile([P, CH], f32r, tag="xsq", name="xsq")
                        nc.scalar.activation(out=xsq, in_=x, func=AF.Square)
                        nc.tensor.matmul(sum_ps[0:1, :], sumw, x,
                                         start=(t == 0), stop=(t == NT - 1))
                        nc.tensor.matmul(sq_ps[0:1, :], sumw, xsq,
                                         start=(t == 0), stop=(t == NT - 1))
                    mean = rowsp.tile([1, CH], f32, tag="rmean", name="mean")
                    nc.scalar.copy(out=mean, in_=sum_ps[0:1, :])
                    t1 = rowsp.tile([1, CH], f32, tag="rt1", name="m2")
                    nc.vector.tensor_mul(t1, mean, mean)
                    var = rowsp.tile([1, CH], f32, tag="rvar", name="var")
                    nc.vector.tensor_tensor(out=var, in0=sq_ps[0:1, :], in1=t1,
                                            op=ALU.subtract)
                    t1b = rowsp.tile([1, CH], f32, tag="rt2", name="lnv")
                    nc.scalar.activation(out=t1b, in_=var, func=AF.Ln,
                                         bias=epsT, scale=1.0)
                    r0 = rowsp.tile([1, CH], f32, tag="rr0", name="r0")
                    nc.scalar.activation(out=r0, in_=t1b, func=AF.Exp, scale=-0.5)
                    # Newton: r = r0*(1.5 - 0.5*var*r0^2)
                    t2 = rowsp.tile([1, CH], f32, tag="rt1", name="r2")
                    nc.vector.tensor_mul(t2, r0, r0)
                    t3 = rowsp.tile([1, CH], f32, tag="rt2", name="tw")
                    nc.vector.tensor_mul(t3, var, t2)
                    t4 = rowsp.tile([1, CH], f32, tag="rt1", name="half")
                    nc.vector.tensor_single_scalar(out=t4, in_=t3, scalar=-0.5,
                                                   op=ALU.mult)
                    t5 = rowsp.tile([1, CH], f32, tag="rt2", name="sN")
                    nc.vector.tensor_single_scalar(out=t5, in_=t4, scalar=1.5,
                                                   op=ALU.add)
                    rr = rowsp.tile([1, CH], f32, tag="rrr", name="rr")
                    nc.vector.tensor_mul(rr, r0, t5)
                    mean_b = bcp.tile([P, CH], f32, tag="meanb", name="meanb")
                    nc.gpsimd.partition_broadcast(mean_b, mean)
                    r_b = bcp.tile([P, CH], f32, tag="rb", name="rb")
                    nc.gpsimd.partition_broadcast(r_b, rr)
                    for t in range(NT):
                        tmp = xsqp.tile([P, CH], f32r, tag="xsq", name="xsub")
                        nc.vector.tensor_tensor(out=tmp, in0=xs[t], in1=mean_b,
                                                op=ALU.subtract)
                        nc.vector.tensor_tensor(
                            out=dst[:, t, c * CH:(c + 1) * CH], in0=tmp, in1=r_b,
                            op=ALU.mult)

            # ---------------- projections ----------------
            def load_w(pool, w_param, t_lo, t_cnt, o_lo, o_cnt, dt):
                tag = f"w{_wctr[0] % 2}"
                _wctr[0] += 1
                wt = pool.tile([P, t_cnt, o_cnt], dt, tag=tag, name="wt")
                nc.sync.dma_start(
                    out=wt, in_=wre(w_param)[:, t_lo:t_lo + t_cnt, o_lo:o_lo + o_cnt])
                return wt

            def proj_dmajor(wp, w_param, bcol_t, XT, dst, n_chunks):
                """dst[:, g, c*CH:...] = (W.T @ X + b), d-major."""
                for quarter in range(4):
                    wt = load_w(wp, w_param, 0, NT, quarter * 256, 256, f32r)
                    for c in range(n_chunks):
                        for dt in range(2):
                            g = quarter * 2 + dt
                            ps = ps_tile("proj")
                            for t in range(NT):
                                nc.tensor.matmul(ps, wt[:, t, dt * P:(dt + 1) * P],
                                                 XT[:, t, c * CH:(c + 1) * CH],
                                                 start=(t == 0), stop=(t == NT - 1))
                            nc.scalar.activation(
                                out=dst[:, g, c * CH:(c + 1) * CH], in_=ps,
                                func=AF.Identity, bias=bcol_t[:, g:g + 1], scale=1.0)

            def proj_v(wp, w_param, b_row_param, XT, Vdst):
                """Vdst[:, tok_t, h, 0:64] = (X @ W + b) tok-major, head-strided."""
                for half in range(2):
                    wt = load_w(wp, w_param, 0, NT, half * CH, CH, f32r)
                    br = constp.tile([1, CH], f32r, tag="brow", bufs=1, name="brow")
                    nc.sync.dma_start(
                        out=br, in_=b_row_param[:, half * CH:(half + 1) * CH])
                    for tok_t in range(NT):
                        ps = ps_tile("projv")
                        for t in range(NT):
                            nc.tensor.matmul(ps, XT[:, t, tok_t * P:(tok_t + 1) * P],
                                             wt[:, t, :], start=(t == 0), stop=False)
                        nc.tensor.matmul(ps, ones512[:, 0:P], br,
                                         start=False, stop=True)
                        nc.scalar.copy(
                            out=Vdst[:, tok_t, half * 8:(half + 1) * 8, 0:HD],
                            in_=ps.rearrange("p (h d) -> p h d", h=8))

            # ---------------- attention ----------------
            def attention(eb_param, KT, QT, Vsb, yT, n_qb, ebp, aep, recbp):
                for hp in range(8):
                    for qb in range(n_qb):
                        ps_y = [ps_tile("y0"), ps_tile("y1")]
                        for ktb in range(2):
                            ae = [aep.tile([P, 4, CH], bf16, tag=f"ae{h}", name="ae")
                                  for h in range(2)]
                            for kt4 in range(4):
                                kt = ktb * 4 + kt4
                                for h in range(2):
                                    ps = ps_tile("att")
                                    nc.tensor.matmul(
                                        ps,
                                        KT[h * HD:(h + 1) * HD, hp, kt * P:(kt + 1) * P],
                                        QT[h * HD:(h + 1) * HD, hp,
                                           qb * CH:(qb + 1) * CH],
                                        start=True, stop=True,
                                        tile_position=(HD * h, 0))
                                    eb_t = ebp.tile([P, CH], bf16, tag="eb", name="ebt")
                                    nc.sync.dma_start(
                                        out=eb_t,
                                        in_=eb_param[2 * hp + h, kt * P:(kt + 1) * P,
                                                     qb * CH:(qb + 1) * CH])
                                    nc.scalar.activation(out=ae[h][:, kt4, :], in_=ps,
                                                         func=AF.Exp)
                                    nc.vector.tensor_tensor(out=ae[h][:, kt4, :],
                                                            in0=ae[h][:, kt4, :],
                                                            in1=eb_t, op=ALU.mult)
                            for h in range(2):
                                for kt4 in range(4):
                                    kt = ktb * 4 + kt4
                                    nc.tensor.matmul(
                                        ps_y[h][0:HD + 1, :],
                                        Vsb[:, kt, 2 * hp + h, :],
                                        ae[h][:, kt4, :],
                                        start=(kt == 0), stop=(kt == NT - 1))
                        for h in range(2):
                            rec = rowsp.tile([1, CH], f32, tag="rmean", name="rec")
                            nc.vector.reciprocal(out=rec, in_=ps_y[h][HD:HD + 1, :])
                            rb = recbp.tile([HD, CH], f32, tag="recb", name="recb")
                            nc.gpsimd.partition_broadcast(rb, rec)
                            nc.vector.tensor_tensor(
                                out=yT[h * HD:(h + 1) * HD, hp, qb * CH:(qb + 1) * CH],
                                in0=ps_y[h][0:HD, :], in1=rb, op=ALU.mult)

            # ---------------- Wo + residual ----------------
            def wo_residual(wp, yT, OUTdst, n_chunks, res_get):
                for quarter in range(4):
                    wt = load_w(wp, wo, 0, NT, quarter * 256, 256, f32r)
                    for c in range(n_chunks):
                        for dt in range(2):
                            g = quarter * 2 + dt
                            ps = ps_tile("wo")
                            for t in range(NT):
                                nc.tensor.matmul(ps, wt[:, t, dt * P:(dt + 1) * P],
                                                 yT[:, t, c * CH:(c + 1) * CH],
                                                 start=(t == 0), stop=(t == NT - 1))
                            res = res_get(g, c)
                            nc.vector.scalar_tensor_tensor(
                                out=OUTdst[:, g, c * CH:(c + 1) * CH],
                                in0=ps, scalar=bcol["bo"][:, g:g + 1], in1=res,
                                op0=ALU.add, op1=ALU.add)

            OUT2 = outerp.tile([P, NT, CH], f32r, name="OUT2")

            # ================= stages 1 & 2 =================
            with tc.tile_pool(name="stg", bufs=1) as stg, \
                 tc.tile_pool(name="aep", bufs=2) as aep, \
                 tc.tile_pool(name="ebp", bufs=2) as ebp, \
                 tc.tile_pool(name="wp", bufs=1) as wp, \
                 tc.tile_pool(name="recbp", bufs=1) as recbp:

                def xt_tile():
                    return stg.tile([P, NT, L], f32r, tag="XT", name="XT")

                # --- stage 1 ---
                XT = xt_tile()
                layernorm(lambda t, c: XT[:, t, c * CH:(c + 1) * CH], XT, 2,
                          dma_src=kT)
                KT1 = stg.tile([P, NT, L], bf16, tag="KT", name="KT1")
                proj_dmajor(wp, wk, bcol["bk"], XT, KT1, 2)

                XT = xt_tile()
                layernorm(lambda t, c: XT[:, t, c * CH:(c + 1) * CH], XT, 2,
                          dma_src=vT)
                Vsb1 = stg.tile([P, NT, H, HD + 1], bf16, tag="Vsb", name="Vsb1")
                nc.sync.dma_start(out=Vsb1[:, :, :, HD:HD + 1], in_=vones[:, :, :, :])
                proj_v(wp, wv, bvr, XT, Vsb1)

                XT = xt_tile()
                layernorm(lambda t, c: XT[:, t, c * CH:(c + 1) * CH], XT, 2,
                          dma_src=qT)
                QT1 = stg.tile([P, NT, L], bf16, tag="QT", name="QT1")
                proj_dmajor(wp, wq, bcol["bq"], XT, QT1, 2)

                yT = xt_tile()
                attention(eb1, KT1, QT1, Vsb1, yT, 2, ebp, aep, recbp)

                OUT = stg.tile([P, NT, L], f32r, tag="OUT", name="OUT")

                def res_s1(g, c):
                    res = xsqp.tile([P, CH], f32r, tag="xsq", name="qres")
                    nc.sync.dma_start(
                        out=res, in_=qT[g * P:(g + 1) * P, c * CH:(c + 1) * CH])
                    return res

                wo_residual(wp, yT, OUT, 2, res_s1)

                # --- stage 2 ---
                XT2 = xt_tile()
                layernorm(lambda t, c: OUT[:, t, c * CH:(c + 1) * CH], XT2, 2)
                KT2 = stg.tile([P, NT, L], bf16, tag="KT", name="KT2")
                proj_dmajor(wp, wk2, bcol["bk2"], XT2, KT2, 2)
                Vsb2 = stg.tile([P, NT, H, HD + 1], bf16, tag="Vsb", name="Vsb2")
                nc.sync.dma_start(out=Vsb2[:, :, :, HD:HD + 1], in_=vones[:, :, :, :])
                proj_v(wp, wv2, bv2r, XT2, Vsb2)
                QT2 = stg.tile([P, NT, CH], bf16, tag="QT", name="QT2")
                proj_dmajor(wp, wq2, bcol["bq2"], XT2, QT2, 1)

                yT2 = xt_tile()
                attention(eb2, KT2, QT2, Vsb2, yT2, 1, ebp, aep, recbp)

                wo_residual(wp, yT2, OUT2, 1,
                            lambda g, c: OUT[:, g, c * CH:(c + 1) * CH])

            # ================= MLP =================
            with tc.tile_pool(name="mlpp", bufs=1) as mlpp, \
                 tc.tile_pool(name="wp2", bufs=1) as wp2:
                XT3 = mlpp.tile([P, NT, CH], f32r, tag="XT3", name="XT3")
                layernorm(lambda t, c: OUT2[:, t, :], XT3, 1)

                h1T = mlpp.tile([P, NT2, CH], f32r, tag="h1T", name="h1T")
                for sb in range(8):
                    for half in range(2):
                        wt = load_w(wp2, w1, 0, NT, sb * CH + half * 256, 256, f32r)
                        for m in range(2):
                            g = sb * 4 + half * 2 + m
                            ps = ps_tile("mlp1")
                            for t in range(NT):
                                nc.tensor.matmul(ps, wt[:, t, m * P:(m + 1) * P],
                                                 XT3[:, t, :], start=(t == 0),
                                                 stop=(t == NT - 1))
                            nc.scalar.activation(out=h1T[:, g, :], in_=ps,
                                                 func=AF.Gelu,
                                                 bias=bcol["b1"][:, g:g + 1],
                                                 scale=1.0)

                h2T = mlpp.tile([P, NT2, CH], f32r, tag="h2T", name="h2T")
                for sb in range(8):
                    pss = [ps_tile("mlp2") for _ in range(4)]
                    for dinb in range(16):
                        wt = load_w(wp2, w2, dinb * 2, 2, sb * CH, CH, f32r)
                        for t in range(2):
                            for m in range(4):
                                nc.tensor.matmul(
                                    pss[m], wt[:, t, m * P:(m + 1) * P],
                                    h1T[:, dinb * 2 + t, :],
                                    start=(dinb == 0 and t == 0),
                                    stop=(dinb == 15 and t == 1))
                    for m in range(4):
                        g = sb * 4 + m
                        nc.scalar.activation(out=h2T[:, g, :], in_=pss[m],
                                             func=AF.Gelu,
                                             bias=bcol["b2"][:, g:g + 1], scale=1.0)

                FIN = mlpp.tile([P, NT, CH], f32r, tag="XT3", name="FIN")
                for doutb in range(2):
                    pss = [ps_tile("mlp3") for _ in range(4)]
                    for dinb in range(16):
                        wt = load_w(wp2, w3, dinb * 2, 2, doutb * CH, CH, f32r)
                        for t in range(2):
                            for m in range(4):
                                nc.tensor.matmul(
                                    pss[m], wt[:, t, m * P:(m + 1) * P],
                                    h2T[:, dinb * 2 + t, :],
                                    start=(dinb == 0 and t == 0),
                                    stop=(dinb == 15 and t == 1))
                    for m in range(4):
                        g = doutb * 4 + m
                        nc.vector.scalar_tensor_tensor(
                            out=FIN[:, g, :], in0=pss[m],
                            scalar=bcol["b3"][:, g:g + 1], in1=OUT2[:, g, :],
                            op0=ALU.add, op1=ALU.add)
                nc.sync.dma_start(out=outT.rearrange("(t p) m -> p t m", p=P),
                                  in_=FIN)

    nc.compile()
    return nc


def _col(b):
    return np.ascontiguousarray(b.reshape(-1, P).T, np.float32)


def _prepare_inputs(inputs):
    f32 = np.float32
    bf = ml_dtypes.bfloat16
    g = {k: np.asarray(v) for k, v in inputs.items()}
    assert np.all(g["attn_mask"] == 1), "kernel assumes all-ones attn_mask"

    def fold(gain, bias, W, b, scale=1.0):
        Wf = (gain[:, None] * W) * scale
        bf_ = (bias @ W + b) * scale
        return np.ascontiguousarray(Wf, f32), bf_.astype(f32)

    Wq, bq = g["Wq"], g["bq"]
    Wk, bk = g["Wk"], g["bk"]
    Wv, bv = g["Wv"], g["bv"]
    sh = {}
    sh["wq"], bq1f = fold(g["g_q"], g["b_q"], Wq, bq, SCALE)
    sh["wk"], bk1f = fold(g["g_k"], g["b_k"], Wk, bk)
    sh["wv"], bv1f = fold(g["g_v"], g["b_v"], Wv, bv)
    sh["wq2"], bq2f = fold(g["g_s"], g["b_s"], Wq, bq, SCALE)
    sh["wk2"], bk2f = fold(g["g_s"], g["b_s"], Wk, bk)
    sh["wv2"], bv2f = fold(g["g_s"], g["b_s"], Wv, bv)
    sh["wo"] = np.ascontiguousarray(g["Wo"], f32)
    sh["w1"], b1f = fold(g["g_2"], g["b_2"], g["W1"], g["b1"])
    sh["w2"] = np.ascontiguousarray(g["W2"], f32)
    sh["w3"] = np.ascontiguousarray(g["W3"], f32)
    sh["bqc"], sh["bkc"] = _col(bq1f), _col(bk1f)
    sh["bq2c"], sh["bk2c"] = _col(bq2f), _col(bk2f)
    sh["boc"] = _col(g["bo"].astype(f32))
    sh["b1c"] = _col(b1f)
    sh["b2c"] = _col(g["b2"].astype(f32))
    sh["b3c"] = _col(g["b3"].astype(f32))
    sh["bvr"] = np.ascontiguousarray(bv1f[None], f32)
    sh["bv2r"] = np.ascontiguousarray(bv2f[None], f32)
    sh["onesr"] = np.ones((1, CH), f32)
    sh["sumwc"] = np.full((P, 1), 1.0 / D, f32)
    sh["vones"] = np.ones((P, NT, H, 1), bf)

    # exp(bias), transposed to [h, k, q]
    ebc = np.exp(g["cross_bias"][:, :L, :L].astype(f32)).transpose(0, 2, 1)
    rel = g["rel_bias"].astype(f32)

    in_maps = []
    for c in range(8):
        b, half = c // 2, c % 2
        if half == 0:
            perm = np.arange(L)
        else:
            perm = np.concatenate([np.arange(CH, L), np.arange(0, CH)])
        m = dict(sh)
        m["qT"] = np.ascontiguousarray(g["query"][b][perm].T, f32)
        m["kT"] = np.ascontiguousarray(g["key"][b][perm].T, f32)
        m["vT"] = np.ascontiguousarray(g["value"][b][perm].T, f32)
        m["eb1"] = np.ascontiguousarray(ebc[:, perm][:, :, perm]).astype(bf)
        idx2 = perm[:, None] - perm[None, :CH] + (MAX - 1)  # [k_local, q_local]
        m["eb2"] = np.exp(rel[:, idx2]).astype(bf)
        in_maps.append(m)
    return in_maps


def kernel(**inputs):
    from concourse.bass_utils import run_bass_kernel_spmd

    if "nc" not in _CACHE:
        _CACHE["nc"] = build_nc()
    nc = _CACHE["nc"]
    in_maps = _prepare_inputs(inputs)
    res = run_bass_kernel_spmd(nc, in_maps, list(range(8)))
    out = np.empty((B, L, D), np.float32)
    for c in range(8):
        b, half = c // 2, c % 2
        out[b, half * CH:(half + 1) * CH, :] = res.results[c]["outT"].T
    return out


# revision 7
# speedup vs baseline: 14720.9625x; 14720.9625x over previous
"""Trainium2 Bass kernel for nn_CrossAttentionBlock (B=4, L=1024, D=1024, H=16).

Sharding: 8 cores = 4 batches x 2 token-halves. Core c handles batch c//2.
Stage 1 (cross-attn) is computed for the full batch on each core (duplicated
within the pair -> no collectives); stage 2 + MLP only for the core's own
512-token half. Per-core token order is permuted so the own half is always
tokens [0:512] locally (keeps the SPMD program identical across cores).

Layout: everything runs "d-major" (features on partitions, tokens on the free
axis), so no on-device transposes are needed anywhere. The host transposes
inputs/outputs and pre-folds LN affine params + attention scale into the
projection weights. Attention biases are applied multiplicatively after exp
(host precomputes exp(bias) in bf16). Projection biases ride the ACT drain's
per-partition bias port.
"""

import sys
import numpy as np

if "/opt/trn_rl_repo" not in sys.path:
    sys.path.insert(0, "/opt/trn_rl_repo")

import ml_dtypes  # noqa: E402

B, L, D, H, MAX = 4, 1024, 1024, 16, 1024
HD = D // H
SCALE = 1.0 / float(np.sqrt(np.float32(HD)))
P = 128
NT = D // P          # 8 d-tiles
CH = 512             # token chunk width
DH = 4 * D           # 4096
NT2 = DH // P        # 32

_CACHE = {}


def build_nc():
    import concourse.bacc as bacc
    import concourse.mybir as mybir
    import concourse.tile as tile

    f32r = mybir.dt.float32r
    f32 = mybir.dt.float32
    bf16 = mybir.dt.bfloat16
    AF = mybir.ActivationFunctionType
    ALU = mybir.AluOpType

    nc = bacc.Bacc("TRN2", target_bir_lowering=False)

    # ---- params ----
    qT = nc.declare_dram_parameter("qT", [D, L], f32r, isOutput=False)
    kT = nc.declare_dram_parameter("kT", [D, L], f32r, isOutput=False)
    vT = nc.declare_dram_parameter("vT", [D, L], f32r, isOutput=False)
    wq = nc.declare_dram_parameter("wq", [D, D], f32r, isOutput=False)
    wk = nc.declare_dram_parameter("wk", [D, D], f32r, isOutput=False)
    wv = nc.declare_dram_parameter("wv", [D, D], f32r, isOutput=False)
    wq2 = nc.declare_dram_parameter("wq2", [D, D], f32r, isOutput=False)
    wk2 = nc.declare_dram_parameter("wk2", [D, D], f32r, isOutput=False)
    wv2 = nc.declare_dram_parameter("wv2", [D, D], f32r, isOutput=False)
    wo = nc.declare_dram_parameter("wo", [D, D], f32r, isOutput=False)
    w1 = nc.declare_dram_parameter("w1", [D, DH], f32r, isOutput=False)
    w2 = nc.declare_dram_parameter("w2", [DH, DH], f32r, isOutput=False)
    w3 = nc.declare_dram_parameter("w3", [DH, D], f32r, isOutput=False)
    # biases, column layout: bcol[p, g] = b[g*128 + p]
    bqc = nc.declare_dram_parameter("bqc", [P, NT], f32, isOutput=False)
    bkc = nc.declare_dram_parameter("bkc", [P, NT], f32, isOutput=False)
    bq2c = nc.declare_dram_parameter("bq2c", [P, NT], f32, isOutput=False)
    bk2c = nc.declare_dram_parameter("bk2c", [P, NT], f32, isOutput=False)
    boc = nc.declare_dram_parameter("boc", [P, NT], f32, isOutput=False)
    b1c = nc.declare_dram_parameter("b1c", [P, NT2], f32, isOutput=False)
    b2c = nc.declare_dram_parameter("b2c", [P, NT2], f32, isOutput=False)
    b3c = nc.declare_dram_parameter("b3c", [P, NT], f32, isOutput=False)
    # V-proj biases as rows (rank-1 matmul path)
    bvr = nc.declare_dram_parameter("bvr", [1, D], f32r, isOutput=False)
    onesr = nc.declare_dram_parameter("onesr", [1, CH], f32r, isOutput=False)
    sumwc = nc.declare_dram_parameter("sumwc", [P, 1], f32r, isOutput=False)
    vones = nc.declare_dram_parameter("vones", [P, NT, H, 1], bf16, isOutput=False)
    bv2r = nc.declare_dram_parameter("bv2r", [1, D], f32r, isOutput=False)
    eb1 = nc.declare_dram_parameter("eb1", [H, L, L], bf16, isOutput=False)
    eb2 = nc.declare_dram_parameter("eb2", [H, L, CH], bf16, isOutput=False)
    outT = nc.declare_dram_parameter("outT", [D, CH], f32r, isOutput=True)

    def wre(w):  # [T*128, N] -> [p, t, n] view for DMA
        return w.rearrange("(t p) o -> p t o", p=P)

    with tile.TileContext(nc) as tc:
        from contextlib import ExitStack
        es = ExitStack()
        with es:
            constp = es.enter_context(tc.tile_pool(name="constp", bufs=1))
            rowsp = es.enter_context(tc.tile_pool(name="rowsp", bufs=1))
            bcp = es.enter_context(tc.tile_pool(name="bcp", bufs=1))
            xsqp = es.enter_context(tc.tile_pool(name="xsqp", bufs=2))
            outerp = es.enter_context(tc.tile_pool(name="outerp", bufs=1))
            psp = es.enter_context(tc.tile_pool(name="psp", bufs=7, space="PSUM"))

            # ---- constants ----
            ones512 = constp.tile([1, CH], f32r)
            nc.sync.dma_start(out=ones512, in_=onesr[:, :])
            sumw = constp.tile([P, 1], f32r)
            nc.sync.dma_start(out=sumw, in_=sumwc[:, :])
            epsT = constp.tile([1, 1], f32)
            nc.vector.memset(epsT, 1e-5)
            bcol = {}
            for nm, prm, g in [("bq", bqc, NT), ("bk", bkc, NT),
                               ("bq2", bq2c, NT), ("bk2", bk2c, NT),
                               ("bo", boc, NT), ("b1", b1c, NT2),
                               ("b2", b2c, NT2), ("b3", b3c, NT)]:
                t = constp.tile([P, g], f32, name=f"bc_{nm}")
                nc.sync.dma_start(out=t, in_=prm[:, :])
                bcol[nm] = t

            _wctr = [0]

            def ps_tile(name):
                return psp.tile([P, CH], f32, tag="mm", name=name)

            # ---------------- LayerNorm ----------------
            def layernorm(x_get, dst, n_chunks, dma_src=None):
                """dst[:, t, c*CH:...] = (x - mean_d(x)) * rsqrt(var_d(x)+eps)."""
                for c in range(n_chunks):
                    xs = []
                    sum_ps = ps_tile("ln_sum")
                    sq_ps = ps_tile("ln_sq")
                    for t in range(NT):
                        x = x_get(t, c)
                        if dma_src is not None:
                            nc.sync.dma_start(
                                out=x, in_=dma_src[t * P:(t + 1) * P, c * CH:(c + 1) * CH])
                        xs.append(x)
                        xsq = xsqp.tile([P, CH], f32r, tag="xsq", name="xsq")
                        nc.scalar.activation(out=xsq, in_=x, func=AF.Square)
                        nc.tensor.matmul(sum_ps[0:1, :], sumw, x,
                                         start=(t == 0), stop=(t == NT - 1))
                        nc.tensor.matmul(sq_ps[0:1, :], sumw, xsq,
                                         start=(t == 0), stop=(t == NT - 1))
                    mean = rowsp.tile([1, CH], f32, tag="rmean", name="mean")
                    nc.scalar.copy(out=mean, in_=sum_ps[0:1, :])
                    t1 = rowsp.tile([1, CH], f32, tag="rt1", name="m2")
                    nc.vector.tensor_mul(t1, mean, mean)
                    var = rowsp.tile([1, CH], f32, tag="rvar", name="var")
                    nc.vector.tensor_tensor(out=var, in0=sq_ps[0:1, :], in1=t1,
                                            op=ALU.subtract)
                    t1b = rowsp.tile([1, CH], f32, tag="rt2", name="lnv")
                    nc.scalar.activation(out=t1b, in_=var, func=AF.Ln,
                                         bias=epsT, scale=1.0)
                    r0 = rowsp.tile([1, CH], f32, tag="rr0", name="r0")
                    nc.scalar.activation(out=r0, in_=t1b, func=AF.Exp, scale=-0.5)
                    # Newton: r = r0*(1.5 - 0.5*var*r0^2)
                    t2 = rowsp.tile([1, CH], f32, tag="rt1", name="r2")
                    nc.vector.tensor_mul(t2, r0, r0)
                    t3 = rowsp.tile([1, CH], f32, tag="rt2", name="tw")
                    nc.vector.tensor_mul(t3, var, t2)
                    t4 = rowsp.tile([1, CH], f32, tag="rt1", name="half")
                    nc.vector.tensor_single_scalar(out=t4, in_=t3, scalar=-0.5,
                                                   op=ALU.mult)
                    t5 = rowsp.tile([1, CH], f32, tag="rt2", name="sN")
                    nc.vector.tensor_single_scalar(out=t5, in_=t4, scalar=1.5,
                                                   op=ALU.add)
                    rr = rowsp.tile([1, CH], f32, tag="rrr", name="rr")
                    nc.vector.tensor_mul(rr, r0, t5)
                    mean_b = bcp.tile([P, CH], f32, tag="meanb", name="meanb")
                    nc.gpsimd.partition_broadcast(mean_b, mean)
                    r_b = bcp.tile([P, CH], f32, tag="rb", name="rb")
                    nc.gpsimd.partition_broadcast(r_b, rr)
                    for t in range(NT):
                        tmp = xsqp.tile([P, CH], f32r, tag="xsq", name="xsub")
                        nc.vector.tensor_tensor(out=tmp, in0=xs[t], in1=mean_b,
                                                op=ALU.subtract)
                        nc.vector.tensor_tensor(
                            out=dst[:, t, c * CH:(c + 1) * CH], in0=tmp, in1=r_b,
                            op=ALU.mult)

            # ---------------- projections ----------------
            def load_w(pool, w_param, t_lo, t_cnt, o_lo, o_cnt, dt):
                tag = f"w{_wctr[0] % 2}"
                _wctr[0] += 1
                wt = pool.tile([P, t_cnt, o_cnt], dt, tag=tag, name="wt")
                nc.sync.dma_start(
                    out=wt, in_=wre(w_param)[:, t_lo:t_lo + t_cnt, o_lo:o_lo + o_cnt])
                return wt

            def proj_dmajor(wp, w_param, bcol_t, XT, dst, n_chunks):
                """dst[:, g, c*CH:...] = (W.T @ X + b), d-major."""
                for quarter in range(4):
                    wt = load_w(wp, w_param, 0, NT, quarter * 256, 256, f32r)
                    for c in range(n_chunks):
                        for dt in range(2):
                            g = quarter * 2 + dt
                            ps = ps_tile("proj")
                            for t in range(NT):
                                nc.tensor.matmul(ps, wt[:, t, dt * P:(dt + 1) * P],
                                                 XT[:, t, c * CH:(c + 1) * CH],
                                                 start=(t == 0), stop=(t == NT - 1))
                            nc.scalar.activation(
                                out=dst[:, g, c * CH:(c + 1) * CH], in_=ps,
                                func=AF.Identity, bias=bcol_t[:, g:g + 1], scale=1.0)

            def proj_v(wp, w_param, b_row_param, XT, Vdst):
                """Vdst[:, tok_t, h, 0:64] = (X @ W + b) tok-major, head-strided."""
                for half in range(2):
                    wt = load_w(wp, w_param, 0, NT, half * CH, CH, f32r)
                    br = constp.tile([1, CH], f32r, tag="brow", bufs=1, name="brow")
                    nc.sync.dma_start(
                        out=br, in_=b_row_param[:, half * CH:(half + 1) * CH])
                    for tok_t in range(NT):
                        ps = ps_tile("projv")
                        for t in range(NT):
                            nc.tensor.matmul(ps, XT[:, t, tok_t * P:(tok_t + 1) * P],
                                             wt[:, t, :], start=(t == 0), stop=False)
                        nc.tensor.matmul(ps, ones512[:, 0:P], br,
                                         start=False, stop=True)
                        nc.scalar.copy(
                            out=Vdst[:, tok_t, half * 8:(half + 1) * 8, 0:HD],
                            in_=ps.rearrange("p (h d) -> p h d", h=8))

            # ---------------- attention ----------------
            def attention(eb_param, KT, QT, Vsb, yT, n_qb, ebp, aep, recbp):
                for hp in range(8):
                    for qb in range(n_qb):
                        ps_y = [ps_tile("y0"), ps_tile("y1")]
                        for ktb in range(2):
                            ae = [aep.tile([P, 4, CH], bf16, tag=f"ae{h}", name="ae")
                                  for h in range(2)]
                            for kt4 in range(4):
                                kt = ktb * 4 + kt4
                                for h in range(2):
                                    ps = ps_tile("att")
                                    nc.tensor.matmul(
                                        ps,
                                        KT[h * HD:(h + 1) * HD, hp, kt * P:(kt + 1) * P],
                                        QT[h * HD:(h + 1) * HD, hp,
                                           qb * CH:(qb + 1) * CH],
                                        start=True, stop=True,
                                        tile_position=(HD * h, 0))
                                    eb_t = ebp.tile([P, CH], bf16, tag="eb", name="ebt")
                                    nc.sync.dma_start(
                                        out=eb_t,
                                        in_=eb_param[2 * hp + h, kt * P:(kt + 1) * P,
                                                     qb * CH:(qb + 1) * CH])
                                    nc.scalar.activation(out=ae[h][:, kt4, :], in_=ps,
                                                         func=AF.Exp)
                                    nc.vector.tensor_tensor(out=ae[h][:, kt4, :],
                                                            in0=ae[h][:, kt4, :],
                                                            in1=eb_t, op=ALU.mult)
                            for h in range(2):
                                for kt4 in range(4):
                                    kt = ktb * 4 + kt4
                                    nc.tensor.matmul(
                                        ps_y[h][0:HD + 1, :],
                                        Vsb[:, kt, 2 * hp + h, :],
                                        ae[h][:, kt4, :],
                                        start=(kt == 0), stop=(kt == NT - 1))
                        for h in range(2):
                            rec = rowsp.tile([1, CH], f32, tag="rmean", name="rec")
                            nc.vector.reciprocal(out=rec, in_=ps_y[h][HD:HD + 1, :])
                            rb = recbp.tile([HD, CH], f32, tag="recb", name="recb")
                            nc.gpsimd.partition_broadcast(rb, rec)
                            nc.vector.tensor_tensor(
                                out=yT[h * HD:(h + 1) * HD, hp, qb * CH:(qb + 1) * CH],
                                in0=ps_y[h][0:HD, :], in1=rb, op=ALU.mult)

            # ---------------- Wo + residual ----------------
            def wo_residual(wp, yT, OUTdst, n_chunks, res_get):
                for quarter in range(4):
                    wt = load_w(wp, wo, 0, NT, quarter * 256, 256, f32r)
                    for c in range(n_chunks):
                        for dt in range(2):
                            g = quarter * 2 + dt
                            ps = ps_tile("wo")
                            for t in range(NT):
                                nc.tensor.matmul(ps, wt[:, t, dt * P:(dt + 1) * P],
                                                 yT[:, t, c * CH:(c + 1) * CH],
                                                 start=(t == 0), stop=(t == NT - 1))
                            res = res_get(g, c)
                            nc.vector.scalar_tensor_tensor(
                                out=OUTdst[:, g, c * CH:(c + 1) * CH],
                                in0=ps, scalar=bcol["bo"][:, g:g + 1], in1=res,
                                op0=ALU.add, op1=ALU.add)

            OUT2 = outerp.tile([P, NT, CH], f32r, name="OUT2")

            # ================= stages 1 & 2 =================
            with tc.tile_pool(name="stg", bufs=1) as stg, \
                 tc.tile_pool(name="aep", bufs=2) as aep, \
                 tc.tile_pool(name="ebp", bufs=2) as ebp, \
                 tc.tile_pool(name="wp", bufs=1) as wp, \
                 tc.tile_pool(name="recbp", bufs=1) as recbp:

                def xt_tile():
                    return stg.tile([P, NT, L], f32r, tag="XT", name="XT")

                # --- stage 1 ---
                XT = xt_tile()
                layernorm(lambda t, c: XT[:, t, c * CH:(c + 1) * CH], XT, 2,
                          dma_src=kT)
                KT1 = stg.tile([P, NT, L], bf16, tag="KT", name="KT1")
                proj_dmajor(wp, wk, bcol["bk"], XT, KT1, 2)

                XT = xt_tile()
                layernorm(lambda t, c: XT[:, t, c * CH:(c + 1) * CH], XT, 2,
                          dma_src=vT)
                Vsb1 = stg.tile([P, NT, H, HD + 1], bf16, tag="Vsb", name="Vsb1")
                nc.sync.dma_start(out=Vsb1[:, :, :, HD:HD + 1], in_=vones[:, :, :, :])
                proj_v(wp, wv, bvr, XT, Vsb1)

                XT = xt_tile()
                layernorm(lambda t, c: XT[:, t, c * CH:(c + 1) * CH], XT, 2,
                          dma_src=qT)
                QT1 = stg.tile([P, NT, L], bf16, tag="QT", name="QT1")
                proj_dmajor(wp, wq, bcol["bq"], XT, QT1, 2)

                yT = xt_tile()
                attention(eb1, KT1, QT1, Vsb1, yT, 2, ebp, aep, recbp)

                OUT = stg.tile([P, NT, L], f32r, tag="OUT", name="OUT")

                def res_s1(g, c):
                    res = xsqp.tile([P, CH], f32r, tag="xsq", name="qres")
                    nc.sync.dma_start(
                        out=res, in_=qT[g * P:(g + 1) * P, c * CH:(c + 1) * CH])
                    return res

                wo_residual(wp, yT, OUT, 2, res_s1)

                # --- stage 2 ---
                XT2 = xt_tile()
                layernorm(lambda t, c: OUT[:, t, c * CH:(c + 1) * CH], XT2, 2)
                KT2 = stg.tile([P, NT, L], bf16, tag="KT", name="KT2")
                proj_dmajor(wp, wk2, bcol["bk2"], XT2, KT2, 2)
                Vsb2 = stg.tile([P, NT, H, HD + 1], bf16, tag="Vsb", name="Vsb2")
                nc.sync.dma_start(out=Vsb2[:, :, :, HD:HD + 1], in_=vones[:, :, :, :])
                proj_v(wp, wv2, bv2r, XT2, Vsb2)
                QT2 = stg.tile([P, NT, CH], bf16, tag="QT", name="QT2")
                proj_dmajor(wp, wq2, bcol["bq2"], XT2, QT2, 1)

                yT2 = xt_tile()
                attention(eb2, KT2, QT2, Vsb2, yT2, 1, ebp, aep, recbp)

                wo_residual(wp, yT2, OUT2, 1,
                            lambda g, c: OUT[:, g, c * CH:(c + 1) * CH])

            # ================= MLP =================
            with tc.tile_pool(name="mlpp", bufs=1) as mlpp, \
                 tc.tile_pool(name="wp2", bufs=1) as wp2:
                XT3 = mlpp.tile([P, NT, CH], f32r, tag="XT3", name="XT3")
                layernorm(lambda t, c: OUT2[:, t, :], XT3, 1)

                h1T = mlpp.tile([P, NT2, CH], f32r, tag="h1T", name="h1T")
                for sb in range(8):
                    for half in range(2):
                        wt = load_w(wp2, w1, 0, NT, sb * CH + half * 256, 256, f32r)
                        for m in range(2):
                            g = sb * 4 + half * 2 + m
                            ps = ps_tile("mlp1")
                            for t in range(NT):
                                nc.tensor.matmul(ps, wt[:, t, m * P:(m + 1) * P],
                                                 XT3[:, t, :], start=(t == 0),
                                                 stop=(t == NT - 1))
                            nc.scalar.activation(out=h1T[:, g, :], in_=ps,
                                                 func=AF.Gelu,
                                                 bias=bcol["b1"][:, g:g + 1],
                                                 scale=1.0)

                h2T = mlpp.tile([P, NT2, CH], f32r, tag="h2T", name="h2T")
                for sb in range(8):
                    pss = [ps_tile("mlp2") for _ in range(4)]
                    for dinb in range(16):
                        wt = load_w(wp2, w2, dinb * 2, 2, sb * CH, CH, f32r)
                        for t in range(2):
                            for m in range(4):
                                nc.tensor.matmul(
                                    pss[m], wt[:, t, m * P:(m + 1) * P],
                                    h1T[:, dinb * 2 + t, :],
                                    start=(dinb == 0 and t == 0),
                                    stop=(dinb == 15 and t == 1))
                    for m in range(4):
                        g = sb * 4 + m
                        nc.scalar.activation(out=h2T[:, g, :], in_=pss[m],
                                             func=AF.Gelu,
                                             bias=bcol["b2"][:, g:g + 1], scale=1.0)

                FIN = mlpp.tile([P, NT, CH], f32r, tag="XT3", name="FIN")
                for doutb in range(2):
                    pss = [ps_tile("mlp3") for _ in range(4)]
                    for dinb in range(16):
                        wt = load_w(wp2, w3, dinb * 2, 2, doutb * CH, CH, f32r)
                        for t in range(2):
                            for m in range(4):
                                nc.tensor.matmul(
                                    pss[m], wt[:, t, m * P:(m + 1) * P],
                                    h2T[:, dinb * 2 + t, :],
                                    start=(dinb == 0 and t == 0),
                                    stop=(dinb == 15 and t == 1))
                    for m in range(4):
                        g = doutb * 4 + m
                        nc.vector.scalar_tensor_tensor(
                            out=FIN[:, g, :], in0=pss[m],
                            scalar=bcol["b3"][:, g:g + 1], in1=OUT2[:, g, :],
                            op0=ALU.add, op1=ALU.add)
                nc.sync.dma_start(out=outT.rearrange("(t p) m -> p t m", p=P),
                                  in_=FIN)

    nc.compile()
    return nc


def _col(b):
    return np.ascontiguousarray(b.reshape(-1, P).T, np.float32)


def _prepare_inputs(inputs):
    f32 = np.float32
    bf = ml_dtypes.bfloat16
    g = {k: np.asarray(v) for k, v in inputs.items()}
    assert np.all(g["attn_mask"] == 1), "kernel assumes all-ones attn_mask"

    def fold(gain, bias, W, b, scale=1.0):
        Wf = (gain[:, None] * W) * scale
        bf_ = (bias @ W + b) * scale
        return np.ascontiguousarray(Wf, f32), bf_.astype(f32)

    Wq, bq = g["Wq"], g["bq"]
    Wk, bk = g["Wk"], g["bk"]
    Wv, bv = g["Wv"], g["bv"]
    sh = {}
    sh["wq"], bq1f = fold(g["g_q"], g["b_q"], Wq, bq, SCALE)
    sh["wk"], bk1f = fold(g["g_k"], g["b_k"], Wk, bk)
    sh["wv"], bv1f = fold(g["g_v"], g["b_v"], Wv, bv)
    sh["wq2"], bq2f = fold(g["g_s"], g["b_s"], Wq, bq, SCALE)
    sh["wk2"], bk2f = fold(g["g_s"], g["b_s"], Wk, bk)
    sh["wv2"], bv2f = fold(g["g_s"], g["b_s"], Wv, bv)
    sh["wo"] = np.ascontiguousarray(g["Wo"], f32)
    sh["w1"], b1f = fold(g["g_2"], g["b_2"], g["W1"], g["b1"])
    sh["w2"] = np.ascontiguousarray(g["W2"], f32)
    sh["w3"] = np.ascontiguousarray(g["W3"], f32)
    sh["bqc"], sh["bkc"] = _col(bq1f), _col(bk1f)
    sh["bq2c"], sh["bk2c"] = _col(bq2f), _col(bk2f)
    sh["boc"] = _col(g["bo"].astype(f32))
    sh["b1c"] = _col(b1f)
    sh["b2c"] = _col(g["b2"].astype(f32))
    sh["b3c"] = _col(g["b3"].astype(f32))
    sh["bvr"] = np.ascontiguousarray(bv1f[None], f32)
    sh["bv2r"] = np.ascontiguousarray(bv2f[None], f32)
    sh["onesr"] = np.ones((1, CH), f32)
    sh["sumwc"] = np.full((P, 1), 1.0 / D, f32)
    sh["vones"] = np.ones((P, NT, H, 1), bf)

    # exp(bias), transposed to [h, k, q]
    ebc = np.exp(g["cross_bias"][:, :L, :L].astype(f32)).transpose(0, 2, 1)
    rel = g["rel_bias"].astype(f32)

    in_maps = []
    for c in range(8):
        b, half = c // 2, c % 2
        if half == 0:
            perm = np.arange(L)
        else:
            perm = np.concatenate([np.arange(CH, L), np.arange(0, CH)])
        m = dict(sh)
        m["qT"] = np.ascontiguousarray(g["query"][b][perm].T, f32)
        m["kT"] = np.ascontiguousarray(g["key"][b][perm].T, f32)
        m["vT"] = np.ascontiguousarray(g["value"][b][perm].T, f32)
        m["eb1"] = np.ascontiguousarray(ebc[:, perm][:, :, perm]).astype(bf)
        idx2 = perm[:, None] - perm[None, :CH] + (MAX - 1)  # [k_local, q_local]
        m["eb2"] = np.exp(rel[:, idx2]).astype(bf)
        in_maps.append(m)
    return in_maps


def kernel(**inputs):
    from concourse.bass_utils import run_bass_kernel_spmd

    if "nc" not in _CACHE:
        _CACHE["nc"] = build_nc()
    nc = _CACHE["nc"]
    in_maps = _prepare_inputs(inputs)
    res = run_bass_kernel_spmd(nc, in_maps, list(range(8)))
    out = np.empty((B, L, D), np.float32)
    for c in range(8):
        b, half = c // 2, c % 2
        out[b, half * CH:(half + 1) * CH, :] = res.results[c]["outT"].T
    return out
